# revision 1
# baseline (speedup 1.0000x reference)
"""Trainium2 Bass kernel for nn_Decode (CenterNet-style polygon decode).

8 NeuronCores SPMD. Device per core: conv3x3(64->256)+relu -> conv1x1(256->64)
in bf16 on the PE (3 shift-pair K=128 matmuls + 3 zero-padded tap matmuls per
512-px PSUM tile, weights stationary), fused ACT epilogues (bias+relu+cast),
plus the full init_polys pipeline (center point math on DVE) and the +1-shifted
conv input copy built on-device via an SWDGE SBUF->SBUF DMA (halves staging;
note: the same copy issued from HWDGE/scalar hard-crashes the exec unit).
Centers sorted by image, block of 128 per core; each core convolves the <=2
images its block touches and returns its primary image's feature plane.
Host: input layout prep, bilinear sampling of the device-computed fp16
feature map, and the fused (fuse_w@poly_w) refine matmul. (The two
device gather primitives are unusable in this container: indirect DMA
silently no-ops under bass2jax/PJRT, dma_gather's Q7 library load fails
codegen outside Bacc; see kernel_full_wip.py for the complete device design.)
"""
import sys
sys.path.insert(0, '/opt/trn_rl_repo')
import numpy as np
import ml_dtypes

import concourse.bass as bass
import concourse.mybir as mybir
import concourse.tile as tile
from concourse import library_config
from concourse.bass_utils import run_bass_kernel_spmd

F32 = mybir.dt.float32
BF16 = mybir.dt.bfloat16
FP16 = mybir.dt.float16
I32 = mybir.dt.int32
ALU = mybir.AluOpType
ACTF = mybir.ActivationFunctionType
BF = ml_dtypes.bfloat16

P = 128
NC = 128
NCORES = 8
B, C, H, W = 4, 64, 128, 128
GRID = 130
NPIX = GRID * GRID          # 16900
IMP = NPIX + 124            # 17024 (%128 == 0)
KDIM = 129 * 64
KPAD = 65 * 128             # 8320
DOWN = 4.0
_cache = {}


def _rework_ap(base_ap, extra_off, dims):
    return bass.AP(tensor=base_ap.tensor, offset=base_ap.offset + extra_off, ap=dims)


def build_nc():
    nc = bass.Bass()
    x_in = nc.dram_tensor("x_in", [64, 2, NPIX], BF16, kind="ExternalInput")
    whrows = nc.dram_tensor("whrows", [128, 2, 256], F32, kind="ExternalInput")
    o_f = nc.dram_tensor("o_f", [64, 1, IMP], FP16, kind="ExternalOutput")
    ctv4 = nc.dram_tensor("ctv4", [128, 2, 129], F32, kind="ExternalInput")
    w1 = nc.dram_tensor("w1", [128, 6, 2, 128], BF16, kind="ExternalInput")
    b1 = nc.dram_tensor("b1", [128, 2], F32, kind="ExternalInput")
    w2 = nc.dram_tensor("w2", [128, 2, 64], BF16, kind="ExternalInput")
    b2 = nc.dram_tensor("b2", [64, 1], F32, kind="ExternalInput")
    o_init = nc.dram_tensor("o_init", [128, 2, 130], F32, kind="ExternalOutput")

    with tile.TileContext(nc) as tc:
        with tc.tile_pool(name="persist", bufs=1) as pp:
            w1_sb = pp.tile([128, 6, 2, 128], BF16)
            b1_sb = pp.tile([128, 2], F32)
            w2_sb = pp.tile([128, 2, 64], BF16)
            b2_sb = pp.tile([64, 1], F32)
            init_t = pp.tile([128, 2, 129], F32)
            scr = pp.tile([128, 4], F32)

            def fence(lanes=True):
                pass

            def safe_store(dram_ap, sbuf_ap, n_touch=1):
                # pre-sync ACT on the producer, then issue the DMA from ACT's
                # sequencer so the DMA itself needs <=1 inline wait.
                t = sbuf_ap
                touch = _rework_ap(t, 0, [[t.ap[0][0], min(t.ap[0][1], 128)], [1, 1]])
                nc.scalar.activation(scr[0:touch.ap[0][1], 0:1], touch, ACTF.Copy)
                nc.scalar.dma_start(dram_ap, sbuf_ap)
            nc.sync.dma_start(w1_sb[:], w1[:])
            nc.sync.dma_start(b1_sb[:], b1[:])
            nc.sync.dma_start(w2_sb[:], w2[:])
            nc.sync.dma_start(b2_sb[:], b2[:])

            # ---------- point math + conv (x_sb, f_sb scoped) ----------
            fpool = tc.tile_pool(name="fpool", bufs=1)
            fpl = fpool.__enter__()
            f_sb = fpl.tile([64, 2, IMP], FP16)
            nc.vector.memset(f_sb[:], 0.0)
            xpool = tc.tile_pool(name="xpool", bufs=1)
            xpl = xpool.__enter__()
            x_sb = xpl.tile([128, 2, NPIX], BF16)
            nc.sync.dma_start(x_sb[0:64, :, :], x_in[:])
            # build the +1-shifted copy in partitions 64:127 on device (SWDGE):
            # dst[64+p, k] = x[p, k+1] for k < 2*NPIX-1; last element zeroed.
            nc.vector.memset(x_sb[64:128, 1, NPIX - 130:NPIX], 0.0)
            xa0 = x_sb[:]
            ps0 = xa0.ap[0][0]
            sh_src = _rework_ap(xa0, 1, [[ps0, 64], [130, 259], [1, 130]])
            sh_dst = _rework_ap(xa0, 64 * ps0, [[ps0, 64], [130, 259], [1, 130]])
            nc.gpsimd.dma_start(sh_dst, sh_src)
            with tc.tile_pool(name="pts", bufs=1) as pt:
                whg = pt.tile([128, 2, 256], F32)
                nc.sync.dma_start(whg[:], whrows[:])
                whv = whg[:, 0:1, :].rearrange("p a (j t) -> p (a t) j", t=2)  # [128,2,128]
                ctv4_sb = pt.tile([128, 2, 129], F32)
                nc.sync.dma_start(ctv4_sb[:], ctv4[:])
                nc.vector.tensor_scalar(init_t[:, :, 0:128], whv, 40.0, None, ALU.mult)
                nc.vector.tensor_tensor(init_t[:, :, 0:128], init_t[:, :, 0:128],
                                        ctv4_sb[:, :, 0:128], ALU.add)
                nc.vector.tensor_copy(init_t[:, :, 128:129], ctv4_sb[:, :, 128:129])
                safe_store(o_init[:, :, 0:129], init_t[:])

                pass
            fence()
            # ---------- conv ----------
            PAIR_BASE = [-131, -1, 129]
            with tc.tile_pool(name="conv", bufs=4) as cp, \
                 tc.tile_pool(name="cps", bufs=2, space="PSUM") as cps, \
                 tc.tile_pool(name="cps2", bufs=2, space="PSUM") as cps2:
                xa = x_sb[:]
                pstep = xa.ap[0][0]
                for img in range(2):
                    for t in range(32):
                        y0r = 4 * t
                        pbase = img * NPIX + (y0r + 1) * GRID + 1
                        f1t = []
                        for half in range(2):
                            ps = cps.tile([128, 512], F32, space="PSUM", tag="c1")
                            first = True
                            for s, db in enumerate(PAIR_BASE):
                                rhs = _rework_ap(xa, pbase + db,
                                                 [[pstep, 128], [GRID, 4], [1, 128]])
                                nc.tensor.matmul(ps[:], w1_sb[:, s, half, :], rhs,
                                                 start=first, stop=False,
                                                 skip_group_check=not first)
                                first = False
                            rhs3 = _rework_ap(xa, pbase - 129,
                                              [[pstep, 128], [GRID, 4], [1, 128]])
                            nc.tensor.matmul(ps[:], w1_sb[:, 3, half, :], rhs3,
                                             start=False, stop=False,
                                             skip_group_check=True)
                            rhs4 = _rework_ap(xa, pbase,
                                              [[pstep, 128], [GRID, 4], [1, 128]])
                            nc.tensor.matmul(ps[:], w1_sb[:, 4, half, :], rhs4,
                                             start=False, stop=False,
                                             skip_group_check=True)
                            rhs5 = _rework_ap(xa, pbase + 131,
                                              [[pstep, 128], [GRID, 4], [1, 128]])
                            nc.tensor.matmul(ps[:], w1_sb[:, 5, half, :], rhs5,
                                             start=False, stop=True,
                                             skip_group_check=True)
                            f1 = cp.tile([128, 512], BF16, tag=f"f1{half}")
                            nc.scalar.activation(f1[:], ps[:], ACTF.Relu,
                                                 bias=b1_sb[:, half:half + 1])
                            f1t.append(f1)
                        ps2 = cps2.tile([64, 512], F32, space="PSUM", tag="c2")
                        nc.tensor.matmul(ps2[:], w2_sb[:, 0, :], f1t[0][:],
                                         start=True, stop=False)
                        nc.tensor.matmul(ps2[:], w2_sb[:, 1, :], f1t[1][:],
                                         start=False, stop=True, skip_group_check=True)
                        fa = f_sb[:]
                        dst = _rework_ap(fa, img * IMP + (y0r + 1) * GRID + 1,
                                         [[fa.ap[0][0], 64], [GRID, 4], [1, 128]])
                        nc.scalar.activation(dst, ps2[:], ACTF.Identity,
                                             bias=b2_sb[:, 0:1])

            xpool.__exit__(None, None, None)
            safe_store(o_f[:], f_sb[:, 0:1, :])
            fpool.__exit__(None, None, None)
    _split_waits(nc)
    return nc


_SEQ_OK = ('InstUnconditionalBranch', 'InstNoOp', 'InstEventSemaphoreOp')


def _split_waits(nc, limit=1):
    """Walrus wait-slot limits: move multi-waits onto injected NoOps."""
    nid = [0]
    for f in nc.m.functions:
        for bb in f.blocks:
            il = bb.instructions
            out = []
            for ins in il:
                si = ins.sync_info
                nm = ins.__class__.__name__
                if (si is not None and len(si.on_wait) > limit
                        and nm not in _SEQ_OK):
                    waits = list(si.on_wait)
                    for k in range(0, len(waits), 1):
                        no = mybir.InstNoOp(name=f"I-wsplit{nid[0]}", ins=[], outs=[])
                        nid[0] += 1
                        no.engine = ins.engine
                        no.sync_info = mybir.SyncInfo(on_wait=waits[k:k + 1], on_update=[])
                        out.append(no)
                    ins.sync_info = mybir.SyncInfo(on_wait=[], on_update=list(si.on_update))
                out.append(ins)
            il[:] = out


def _prep(inputs):
    cnn = np.asarray(inputs['cnn_feature'], np.float32)
    wh = np.asarray(inputs['wh_pred'], np.float32)
    w1 = np.asarray(inputs['conv1_w'], np.float32)
    b1 = np.asarray(inputs['conv1_b'], np.float32)
    w2 = np.asarray(inputs['conv2_w'], np.float32)
    b2 = np.asarray(inputs['conv2_b'], np.float32)
    pw = np.asarray(inputs['poly_w'], np.float32)
    fw = np.asarray(inputs['fuse_w'], np.float32)
    fb = np.asarray(inputs['fuse_b'], np.float32)
    ct_ind = np.asarray(inputs['ct_ind'], np.int32)
    ct_img = np.asarray(inputs['ct_img_idx'], np.int32)

    order = np.argsort(ct_img, kind='stable')
    # fused refine weights; device sample col j: j<128 -> ref sample j+1, j=128 -> ref 0

    w1r = w1.reshape(256, 64, 3, 3)
    def tapw(dy, dx):
        return w1r[:, :, dy + 1, dx + 1]             # [256, 64]
    w1_dev = np.zeros((128, 6, 2, 128), np.float32)
    pairs = [((-1, -1), (-1, 0)), ((0, -1), (0, 0)), ((1, -1), (1, 0))]
    for s, (ta, tb) in enumerate(pairs):
        for half in range(2):
            w1_dev[0:64, s, half, :] = tapw(*ta)[128 * half:128 * (half + 1)].T
            w1_dev[64:128, s, half, :] = tapw(*tb)[128 * half:128 * (half + 1)].T
    for half in range(2):
        w1_dev[0:64, 3, half, :] = tapw(-1, 1)[128 * half:128 * (half + 1)].T
        w1_dev[64:128, 4, half, :] = tapw(0, 1)[128 * half:128 * (half + 1)].T
        w1_dev[0:64, 5, half, :] = tapw(1, 1)[128 * half:128 * (half + 1)].T
    w2t = w2.reshape(64, 256).T                      # [256, 64]
    w2_dev = np.ascontiguousarray(np.stack([w2t[0:128], w2t[128:256]], axis=1))

    in_maps, metas = [], []
    for c in range(NCORES):
        slots = order[c * NC:(c + 1) * NC]
        imgs = sorted(set(ct_img[slots].tolist()))
        imgA = imgs[0]
        imgB = imgs[1] if len(imgs) > 1 else imgs[0]
        isel = (ct_img[slots] == imgB).astype(np.int32)

        def padimg(b):
            g = np.zeros((64, GRID, GRID), np.float32)
            g[:, 1:129, 1:129] = cnn[b]
            return g.reshape(64, NPIX)
        x_dev = np.stack([padimg(imgA), padimg(imgB)], axis=1).astype(BF)  # [64,2,NPIX]

        whr = wh[ct_img[slots], :, ct_ind[slots] // W, ct_ind[slots] % W]
        whrows_h = np.zeros((128, 2, 256), np.float32)
        whrows_h[:, 0, :] = whr
        ctx = (ct_ind[slots] % W).astype(np.float32)
        cty = (ct_ind[slots] // W).astype(np.float32)
        ctv_h = np.zeros((128, 2, 129), np.float32)
        ctv_h[:, 0, :] = ctx[:, None]
        ctv_h[:, 1, :] = cty[:, None]
        in_maps.append({
            'x_in': x_dev,
            'whrows': whrows_h,
            'ctv4': 4.0 * ctv_h,
            'w1': w1_dev.astype(BF),
            'b1': np.stack([b1[0:128], b1[128:256]], 1).astype(np.float32),
            'w2': w2_dev.astype(BF),
            'b2': b2.reshape(64, 1).astype(np.float32),
        })
        metas.append(slots)
        _cache.setdefault('core_imgs', {})[c] = (imgA, imgB)
    return in_maps, metas


def _bilinear(feature, points, img_idx, h, w):
    # feature [B, C, H, W] -> flat [B*H*W, C]; one batched 4-neighbor gather
    Bn, Cc = feature.shape[0], feature.shape[1]
    f_flat = np.ascontiguousarray(feature.transpose(0, 2, 3, 1)).reshape(-1, Cc)
    x = points[..., 0]
    y = points[..., 1]
    x0 = np.floor(x)
    y0 = np.floor(y)
    wx = x - x0
    wy = y - y0
    x0i = x0.astype(np.int64)
    y0i = y0.astype(np.int64)
    base = (img_idx.astype(np.int64) * h)[:, None]
    idx = np.empty((4,) + x.shape, np.int64)
    wts = np.empty((4,) + x.shape, np.float32)
    k = 0
    for dy in range(2):
        for dx in range(2):
            yi = y0i + dy
            xi = x0i + dx
            valid = (xi >= 0) & (xi < w) & (yi >= 0) & (yi < h)
            idx[k] = (base + np.clip(yi, 0, h - 1)) * w + np.clip(xi, 0, w - 1)
            wts[k] = ((wx if dx else 1.0 - wx) * (wy if dy else 1.0 - wy)
                      * valid.astype(np.float32))
            k += 1
    g = f_flat[idx.reshape(-1)].reshape(idx.shape + (Cc,))
    return np.einsum('knj,knjc->njc', wts, g, optimize=True)


def kernel(**inputs):
    in_maps, metas = _prep(inputs)
    if 'nc' not in _cache:
        _cache['nc'] = build_nc()
    res = run_bass_kernel_spmd(_cache['nc'], in_maps, core_ids=list(range(NCORES)))
    ct_ind = np.asarray(inputs['ct_ind'], np.int64)
    ct_img = np.asarray(inputs['ct_img_idx'], np.int64)
    N = ct_ind.shape[0]
    init = np.zeros((N, P, 2), np.float32)
    for c in range(NCORES):
        init[metas[c]] = res.results[c]['o_init'][:, :, 0:128].transpose(0, 2, 1)

    # assemble device-computed f (fp16); cores return their imgA plane only
    f_dev = np.zeros((B, 64, H, W), np.float32)
    have = set()
    for c in range(NCORES):
        b = _cache.setdefault('core_imgs', {}).get(c, (None,))[0]
        if b is None or b in have:
            continue
        fa = res.results[c]['o_f'].astype(np.float32)     # [64, 1, IMP]
        f_dev[b] = fa[:, 0, :NPIX].reshape(64, GRID, GRID)[:, 1:129, 1:129]
        have.add(b)
    need = set(np.unique(ct_img).tolist()) - have
    if need:                                  # rare: image never an imgA
        cnn = np.asarray(inputs['cnn_feature'], np.float32)
        w1f = np.asarray(inputs['conv1_w'], np.float32)
        b1f = np.asarray(inputs['conv1_b'], np.float32)
        w2f = np.asarray(inputs['conv2_w'], np.float32)
        b2f = np.asarray(inputs['conv2_b'], np.float32)
        for b in sorted(need):
            xp = np.zeros((64, H + 2, W + 2), np.float32)
            xp[:, 1:-1, 1:-1] = cnn[b]
            cols = np.empty((H * W, 64 * 9), np.float32)
            k = 0
            for dy in range(3):
                for dx in range(3):
                    cols[:, k * 64:(k + 1) * 64] = (
                        xp[:, dy:dy + H, dx:dx + W].reshape(64, H * W).T)
                    k += 1
            wm = w1f.transpose(2, 3, 1, 0).reshape(9 * 64, 256)
            ff = np.maximum(cols @ wm + b1f, 0.0)
            ff = ff @ w2f.reshape(64, 256).T + b2f
            f_dev[b] = ff.T.reshape(64, H, W)
            have.add(b)

    # host refine using device f
    ct = np.stack([ct_ind % W, ct_ind // W], -1).astype(np.float32)
    init_polys = init / DOWN if False else None
    ip = np.asarray(init, np.float32) / 4.0               # init_polys (pre-DOWN)
    points = np.concatenate([ct[:, None, :], ip], axis=1)
    fp = _bilinear(f_dev, points, ct_img, H, W)
    fp = fp.transpose(0, 2, 1).reshape(N, -1)
    fw = np.asarray(inputs['fuse_w'], np.float32)
    fb = np.asarray(inputs['fuse_b'], np.float32)
    Wf = _cache.get('Wf')
    if Wf is None:
        Wf = _cache['Wf'] = (fw @ np.asarray(inputs['poly_w'], np.float32)).T
    offsets = (fp @ Wf + fb).reshape(N, P, 2)
    coar = offsets * 4.0 * 4.0 + init
    return init, coar



# revision 3
# speedup vs baseline: 2.9831x; 2.9831x over previous
"""Trainium2 Bass kernel for nn_Decode (CenterNet-style polygon decode).

8 NeuronCores SPMD, half-image per core (core c: image c//2, output rows
64*(c%2) .. 64*(c%2)+64). Device does the conv stack: conv3x3(64->256)+relu
-> conv1x1(256->64), in bf16 on the PE. conv1 uses the shift-pair trick
(3 K=128 pair matmuls + 3 zero-padded tap matmuls per 512-px PSUM tile,
weights stationary); conv2 is PE-transposed (f1 chunk stationary, w2 moving)
so PSUM comes out [px, ch] and the output slab lands in DRAM as
(rows*131, 64) fp16 inside a zero-padded plane -- the host bilinear then
needs no transpose and no validity masks (zero border == padding_mode=zeros).

Host: init-poly math (trivial), pair-gather bilinear off the padded fp16
plane, fused refine matmul (fp @ (fuse_w@poly_w) reordered j-major).

Transfer schedule per call (axon link ~60-75MB/s is the bottleneck):
x slabs 8.8MB up + f slabs 8.6MB down; weights are cached on device
(byte-compared against the previous call); donated output zero-buffers are
created on device, never uploaded. Host index/init math overlaps exec.
"""
import sys
sys.path.insert(0, '/opt/trn_rl_repo')
import numpy as np
import ml_dtypes

import jax
import jax.numpy as jnp
from jax.experimental.shard_map import shard_map
from jax.sharding import Mesh, PartitionSpec, NamedSharding

import concourse.bass as bass
import concourse.mybir as mybir
import concourse.tile as tile
from concourse.bass2jax import _bass_exec_p, partition_id_tensor, install_neuronx_cc_hook

F32 = mybir.dt.float32
BF16 = mybir.dt.bfloat16
FP16 = mybir.dt.float16
ALU = mybir.AluOpType
ACTF = mybir.ActivationFunctionType
BF = ml_dtypes.bfloat16

P = 128
NCORES = 8
B, C, H, W = 4, 64, 128, 128
ROWS = 64                 # output rows per core
INROWS = ROWS + 2         # input rows incl. halo
GRID = 130                # input slab width (1px pad each side)
OGRID = 131               # output plane width (1 left pad, 2 right pad)
NPIXC = INROWS * GRID     # 8580 input px per core
OROWS = ROWS * OGRID      # 8384 output rows per core
_cache = {}


def _rework_ap(base_ap, extra_off, dims):
    return bass.AP(tensor=base_ap.tensor, offset=base_ap.offset + extra_off, ap=dims)


def build_nc():
    nc = bass.Bass()
    x_in = nc.dram_tensor("x_in", [64, INROWS, GRID], BF16, kind="ExternalInput")
    w1 = nc.dram_tensor("w1", [128, 6, 2, 128], BF16, kind="ExternalInput")
    b1 = nc.dram_tensor("b1", [128, 2], F32, kind="ExternalInput")
    w2 = nc.dram_tensor("w2", [128, 2, 64], BF16, kind="ExternalInput")
    b2 = nc.dram_tensor("b2", [128, 64], F32, kind="ExternalInput")
    o_f = nc.dram_tensor("o_f", [OROWS, 64], FP16, kind="ExternalOutput")

    with tile.TileContext(nc) as tc:
        with tc.tile_pool(name="persist", bufs=1) as pp:
            w1_sb = pp.tile([128, 6, 2, 128], BF16)
            b1_sb = pp.tile([128, 2], F32)
            w2_sb = pp.tile([128, 2, 64], BF16)
            b2_sb = pp.tile([128, 64], F32)
            x_sb = pp.tile([128, NPIXC], BF16)
            nc.sync.dma_start(w1_sb[:], w1[:])
            nc.sync.dma_start(b1_sb[:], b1[:])
            nc.sync.dma_start(w2_sb[:], w2[:])
            nc.sync.dma_start(b2_sb[:], b2[:])
            nc.sync.dma_start(x_sb[0:64, :], x_in[:])
            # build the +1-shifted copy in partitions 64:127 on device (SWDGE;
            # the same copy issued from HWDGE/scalar hard-crashes the exec unit).
            # shifted[64+p, k] = x[p, k+1] for k in [0, NPIXC-1).
            nc.vector.memset(x_sb[64:128, NPIXC - 4:NPIXC], 0.0)
            xa0 = x_sb[:]
            ps0 = xa0.ap[0][0]
            sh_src = _rework_ap(xa0, 1, [[ps0, 64], [1, NPIXC - 1]])
            sh_dst = _rework_ap(xa0, 64 * ps0, [[ps0, 64], [1, NPIXC - 1]])
            nc.gpsimd.dma_start(sh_dst, sh_src)

            PAIR_BASE = [-131, -1, 129]      # dy*130 - 1 for dy = -1,0,1
            with tc.tile_pool(name="conv", bufs=4) as cp, \
                 tc.tile_pool(name="slab", bufs=2) as sp, \
                 tc.tile_pool(name="cps", bufs=2, space="PSUM") as cps, \
                 tc.tile_pool(name="cps2", bufs=4, space="PSUM") as cps2:
                xa = x_sb[:]
                pstep = xa.ap[0][0]
                for t in range(16):
                    y0r = 4 * t
                    pbase = (y0r + 1) * GRID + 1
                    f1t = []
                    for half in range(2):
                        ps = cps.tile([128, 512], F32, space="PSUM", tag="c1")
                        first = True
                        for s, db in enumerate(PAIR_BASE):
                            rhs = _rework_ap(xa, pbase + db,
                                             [[pstep, 128], [GRID, 4], [1, 128]])
                            nc.tensor.matmul(ps[:], w1_sb[:, s, half, :], rhs,
                                             start=first, stop=False,
                                             skip_group_check=not first)
                            first = False
                        rhs3 = _rework_ap(xa, pbase - 129,
                                          [[pstep, 128], [GRID, 4], [1, 128]])
                        nc.tensor.matmul(ps[:], w1_sb[:, 3, half, :], rhs3,
                                         start=False, stop=False,
                                         skip_group_check=True)
                        rhs4 = _rework_ap(xa, pbase,
                                          [[pstep, 128], [GRID, 4], [1, 128]])
                        nc.tensor.matmul(ps[:], w1_sb[:, 4, half, :], rhs4,
                                         start=False, stop=False,
                                         skip_group_check=True)
                        rhs5 = _rework_ap(xa, pbase + 131,
                                          [[pstep, 128], [GRID, 4], [1, 128]])
                        nc.tensor.matmul(ps[:], w1_sb[:, 5, half, :], rhs5,
                                         start=False, stop=True,
                                         skip_group_check=True)
                        f1 = cp.tile([128, 512], BF16, tag=f"f1{half}")
                        nc.scalar.activation(f1[:], ps[:], ACTF.Relu,
                                             bias=b1_sb[:, half:half + 1])
                        f1t.append(f1)
                    # conv2, PE-transposed: out[px, ch] per 128-px (=1 row) chunk
                    slab = sp.tile([128, 4, 64], FP16, tag="slab")
                    for m in range(4):
                        ps2 = cps2.tile([128, 64], F32, space="PSUM", tag="c2")
                        nc.tensor.matmul(ps2[:], f1t[0][:, 128 * m:128 * (m + 1)],
                                         w2_sb[:, 0, :], start=True, stop=False)
                        nc.tensor.matmul(ps2[:], f1t[1][:, 128 * m:128 * (m + 1)],
                                         w2_sb[:, 1, :], start=False, stop=True,
                                         skip_group_check=True)
                        nc.vector.tensor_tensor(slab[:, m, :], ps2[:], b2_sb[:],
                                                ALU.add)
                    dst = bass.AP(tensor=o_f, offset=(y0r * OGRID + 1) * 64,
                                  ap=[[64, 128], [OGRID * 64, 4], [1, 64]])
                    nc.sync.dma_start(dst, slab[:])
    _split_waits(nc)
    return nc


_SEQ_OK = ('InstUnconditionalBranch', 'InstNoOp', 'InstEventSemaphoreOp')


def _split_waits(nc, limit=1):
    """Walrus wait-slot limits: move multi-waits onto injected NoOps."""
    nid = [0]
    for f in nc.m.functions:
        for bb in f.blocks:
            il = bb.instructions
            out = []
            for ins in il:
                si = ins.sync_info
                nm = ins.__class__.__name__
                if (si is not None and len(si.on_wait) > limit
                        and nm not in _SEQ_OK):
                    waits = list(si.on_wait)
                    for k in range(0, len(waits), 1):
                        no = mybir.InstNoOp(name=f"I-wsplit{nid[0]}", ins=[], outs=[])
                        nid[0] += 1
                        no.engine = ins.engine
                        no.sync_info = mybir.SyncInfo(on_wait=waits[k:k + 1], on_update=[])
                        out.append(no)
                    ins.sync_info = mybir.SyncInfo(on_wait=[], on_update=list(si.on_update))
                out.append(ins)
            il[:] = out


def _weight_layouts(w1, b1, w2, b2):
    w1r = np.asarray(w1, np.float32).reshape(256, 64, 3, 3)

    def tapw(dy, dx):
        return w1r[:, :, dy + 1, dx + 1]             # [256, 64]
    w1_dev = np.zeros((128, 6, 2, 128), np.float32)
    pairs = [((-1, -1), (-1, 0)), ((0, -1), (0, 0)), ((1, -1), (1, 0))]
    for s, (ta, tb) in enumerate(pairs):
        for half in range(2):
            w1_dev[0:64, s, half, :] = tapw(*ta)[128 * half:128 * (half + 1)].T
            w1_dev[64:128, s, half, :] = tapw(*tb)[128 * half:128 * (half + 1)].T
    for half in range(2):
        w1_dev[0:64, 3, half, :] = tapw(-1, 1)[128 * half:128 * (half + 1)].T
        w1_dev[64:128, 4, half, :] = tapw(0, 1)[128 * half:128 * (half + 1)].T
        w1_dev[0:64, 5, half, :] = tapw(1, 1)[128 * half:128 * (half + 1)].T
    w2t = np.asarray(w2, np.float32).reshape(64, 256).T
    w2_dev = np.ascontiguousarray(np.stack([w2t[0:128], w2t[128:256]], axis=1))
    b1_dev = np.ascontiguousarray(
        np.stack([b1[0:128], b1[128:256]], 1).astype(np.float32))
    b2_dev = np.ascontiguousarray(
        np.broadcast_to(np.asarray(b2, np.float32)[None, :], (128, 64)))
    return (w1_dev.astype(BF), b1_dev, w2_dev.astype(BF), b2_dev)


def _get_rt():
    rt = _cache.get('rt')
    if rt is not None:
        return rt
    install_neuronx_cc_hook()
    nc = build_nc()
    partition_name = nc.partition_id_tensor.name if nc.partition_id_tensor else None
    in_names, out_names, out_avals, zero_shapes = [], [], [], []
    for alloc in nc.m.functions[0].allocations:
        if not isinstance(alloc, mybir.MemoryLocationSet):
            continue
        name = alloc.memorylocations[0].name
        if alloc.kind == "ExternalInput":
            if name != partition_name:
                in_names.append(name)
        elif alloc.kind == "ExternalOutput":
            shape = tuple(alloc.tensor_shape)
            dtype = mybir.dt.np(alloc.dtype)
            out_names.append(name)
            out_avals.append(jax.core.ShapedArray(shape, dtype))
            zero_shapes.append((shape, dtype))
    n_params = len(in_names)
    n_outs = len(out_avals)
    in_names_all = in_names + out_names + ([partition_name] if partition_name else [])
    donate = tuple(range(n_params, n_params + n_outs))

    def _body(*args):
        operands = list(args)
        if partition_name is not None:
            operands.append(partition_id_tensor())
        outs = _bass_exec_p.bind(
            *operands, out_avals=tuple(out_avals),
            in_names=tuple(in_names_all), out_names=tuple(out_names),
            lowering_input_output_aliases=(), sim_require_finite=True,
            sim_require_nnan=True, nc=nc)
        return tuple(outs)

    devices = jax.devices()[:NCORES]
    mesh = Mesh(np.asarray(devices), ("core",))
    sh = NamedSharding(mesh, PartitionSpec("core"))
    sharded = jax.jit(
        shard_map(_body, mesh=mesh,
                  in_specs=(PartitionSpec("core"),) * (n_params + n_outs),
                  out_specs=(PartitionSpec("core"),) * n_outs, check_rep=False),
        donate_argnums=donate, keep_unused=True)
    zeros_fns = [
        jax.jit(lambda s=s, dt=dt: jnp.zeros((NCORES * s[0], *s[1:]), dt),
                out_shardings=sh)
        for s, dt in zero_shapes]
    rt = dict(nc=nc, in_names=in_names, out_names=out_names, sharded=sharded,
              zeros_fns=zeros_fns, sh=sh)
    _cache['rt'] = rt
    return rt


def _dev_weights(rt, inputs):
    """Device-resident per-core-concat weight arrays, re-uploaded only when
    the host bytes change."""
    w1 = np.asarray(inputs['conv1_w'], np.float32)
    b1 = np.asarray(inputs['conv1_b'], np.float32)
    w2 = np.asarray(inputs['conv2_w'], np.float32)
    b2 = np.asarray(inputs['conv2_b'], np.float32)
    cached = _cache.get('wts')
    if cached is not None:
        ow1, ob1, ow2, ob2, dev = cached
        if (np.array_equal(w1, ow1) and np.array_equal(b1, ob1)
                and np.array_equal(w2, ow2) and np.array_equal(b2, ob2)):
            return dev
    lay = _weight_layouts(w1, b1, w2, b2)
    dev = {}
    names = ['w1', 'b1', 'w2', 'b2']
    for name, arr in zip(names, lay):
        cat = np.ascontiguousarray(
            np.broadcast_to(arr[None], (NCORES,) + arr.shape)
        ).reshape(NCORES * arr.shape[0], *arr.shape[1:])
        dev[name] = jax.device_put(cat, rt['sh'])
    for a in dev.values():
        a.block_until_ready()
    _cache['wts'] = (w1.copy(), b1.copy(), w2.copy(), b2.copy(), dev)
    return dev


def _build_x(cnn):
    """(8*64, INROWS, GRID) bf16 slab stack: core c = image c//2,
    input rows 64*(c%2)-1 .. 64*(c%2)+65 (clipped), 1px zero pad cols."""
    X = np.zeros((B, 2, 64, INROWS, GRID), np.float32)
    X[:, 0, :, 1:, 1:129] = cnn[:, :, 0:65, :]       # top half: rows -1..64
    X[:, 1, :, 0:65, 1:129] = cnn[:, :, 63:128, :]   # bottom half: rows 63..128
    return X.reshape(NCORES * 64, INROWS, GRID).astype(BF)


def _bilinear_refine(F_pad, points, img_idx, Wf2, fb, init):
    """points (N,129,2) pixel coords; F_pad (B,130,131,64) fp16 zero-border.
    Returns coarse polys (pre-DOWN refine already folded)."""
    N = points.shape[0]
    x = points[..., 0] - 0.5
    y = points[..., 1] - 0.5
    x0 = np.floor(x)
    y0 = np.floor(y)
    wx = (x - x0).astype(np.float32)[..., None]
    wy = (y - y0).astype(np.float32)[..., None]
    x0i = x0.astype(np.int32)
    y0i = y0.astype(np.int32)
    # padded col of the left neighbor; (129,130) is an all-zero pair, used
    # for fully-OOB x. rows: plain clip works (rows 0 and 129 both zero).
    xsel = np.where(x0i >= -1, np.minimum(x0i + 1, 129), 129)
    ybase = img_idx.astype(np.int32)[:, None] * 130
    y0sel = (ybase + np.clip(y0i + 1, 0, 129)) * 131 + xsel
    y1sel = (ybase + np.clip(y0i + 2, 0, 129)) * 131 + xsel
    flat = F_pad.reshape(-1)
    V = np.lib.stride_tricks.as_strided(
        flat, (B * 130 * 131 - 1, 128), (128, 2), writeable=False)
    P0 = V[y0sel]                       # (N,129,128) fp16
    P1 = V[y1sel]
    A = P0 * (1.0 - wy) + P1 * wy       # f32 (N,129,128)
    fp = A[:, :, :64] * (1.0 - wx) + A[:, :, 64:] * wx
    offsets = fp.reshape(N, 129 * 64) @ Wf2 + fb
    return offsets.reshape(N, P, 2) * 16.0 + init


def kernel(**inputs):
    rt = _get_rt()
    dev_w = _dev_weights(rt, inputs)

    fw = np.asarray(inputs['fuse_w'], np.float32)
    pw = np.asarray(inputs['poly_w'], np.float32)
    cw = _cache.get('Wf2')
    if cw is None or not (np.array_equal(fw, cw[0]) and np.array_equal(pw, cw[1])):
        Wf = (fw @ pw).T                               # (8256, 256) rows c*129+j
        Wf2 = np.ascontiguousarray(
            Wf.reshape(64, 129, 256).transpose(1, 0, 2).reshape(129 * 64, 256))
        _cache['Wf2'] = (fw.copy(), pw.copy(), Wf2)
    Wf2 = _cache['Wf2'][2]

    cnn = np.asarray(inputs['cnn_feature'], np.float32)
    Xc = _build_x(cnn)

    args = [Xc, dev_w['w1'], dev_w['b1'], dev_w['w2'], dev_w['b2']]
    zeros = [fn() for fn in rt['zeros_fns']]
    out_arrs = rt['sharded'](*args, *zeros)            # async dispatch

    # ---- host work overlapped with device exec ----
    wh = np.asarray(inputs['wh_pred'], np.float32)
    ct_ind = np.asarray(inputs['ct_ind'], np.int64)
    ct_img = np.asarray(inputs['ct_img_idx'], np.int64)
    N = ct_ind.shape[0]
    ctx = (ct_ind % W).astype(np.float32)
    cty = (ct_ind // W).astype(np.float32)
    whr = wh[ct_img, :, ct_ind // W, ct_ind % W]       # (N, 2P)
    ct4 = np.stack([ctx, cty], -1) * 4.0               # (N,2)
    init = whr.reshape(N, P, 2) * 40.0 + ct4[:, None, :]
    ct = np.stack([ctx, cty], -1)
    points = np.concatenate([ct[:, None, :], init / 4.0], axis=1)  # (N,129,2)
    fb = np.asarray(inputs['fuse_b'], np.float32)

    # ---- collect f and assemble padded planes ----
    o_f = np.asarray(out_arrs[0])                      # (8*OROWS, 64) fp16
    slabs = o_f.reshape(B, 2, ROWS, OGRID, 64)
    F_pad = np.zeros((B, 130, OGRID, 64), np.float16)
    F_pad[:, 1:65] = slabs[:, 0]
    F_pad[:, 65:129] = slabs[:, 1]

    coar = _bilinear_refine(F_pad, points, ct_img, Wf2, fb, init)
    return init, coar


# revision 7
# speedup vs baseline: 5.4896x; 1.8402x over previous
"""Trainium2 Bass kernel for nn_Decode (CenterNet-style polygon decode).

Single NeuronCore does the conv stack for all 4 images: conv3x3(64->256)+relu
-> conv1x1(256->64) in bf16 on the PE. The axon link (~60-75MB/s, ~50ms fixed
per exec RPC) is the bottleneck, not compute (<1ms on PE), so the design
minimizes bytes moved and RPC count:
  - input goes up unpadded as one contiguous bf16 cast of cnn_feature (8.4MB);
    zero-padding happens on device (memset + strided DMA into SBUF).
  - conv1 uses the shift-pair trick (3 K=128 pair matmuls + 3 zero-padded tap
    matmuls per 512-px PSUM tile, stationary weights); the +1-shifted input
    copy is built on device via an SWDGE SBUF->SBUF DMA (HWDGE/scalar issue
    of the same copy hard-crashes the exec unit).
  - conv2 is PE-transposed (f1 chunk stationary, w2 moving) so PSUM comes out
    [px, ch] and lands in DRAM as 4 zero-padded fp16 planes (130x131, 1px
    left / 2px right pad) -- the host bilinear then needs no transpose and no
    validity masks (zero border == padding_mode=zeros), 8.7MB down.
  - weights are cached on device across calls (byte-compared); donated output
    zero-buffers are created on device, never uploaded.
Host: init-poly math (overlapped with device exec), fused XLA-CPU pair-gather
bilinear off the padded planes, refine matmul via fused (fuse_w@poly_w)
reordered j-major so no transpose of the sampled features is needed.
"""
import sys
sys.path.insert(0, '/opt/trn_rl_repo')
import numpy as np
import ml_dtypes
from functools import partial

import jax
import jax.numpy as jnp

import concourse.bass as bass
import concourse.mybir as mybir
import concourse.tile as tile
from concourse.bass2jax import _bass_exec_p, partition_id_tensor, install_neuronx_cc_hook

F32 = mybir.dt.float32
BF16 = mybir.dt.bfloat16
FP16 = mybir.dt.float16
ALU = mybir.AluOpType
ACTF = mybir.ActivationFunctionType
BF = ml_dtypes.bfloat16

P = 128
B, C, H, W = 4, 64, 128, 128
GRID = 130                 # padded input plane width/height
NPIX = GRID * GRID         # 16900 input px per image
OGRID = 131                # output plane width (1 left pad, 2 right pad)
OPLANE = 130 * OGRID       # 17030 output rows per image
_cache = {}


def _rework_ap(base_ap, extra_off, dims):
    return bass.AP(tensor=base_ap.tensor, offset=base_ap.offset + extra_off, ap=dims)


def build_nc():
    nc = bass.Bass()
    x_in = nc.dram_tensor("x_in", [B, C, H, W], BF16, kind="ExternalInput")
    w1 = nc.dram_tensor("w1", [128, 6, 2, 128], BF16, kind="ExternalInput")
    b1 = nc.dram_tensor("b1", [128, 2], F32, kind="ExternalInput")
    w2 = nc.dram_tensor("w2", [128, 2, 64], BF16, kind="ExternalInput")
    b2 = nc.dram_tensor("b2", [128, 64], F32, kind="ExternalInput")
    o_f = nc.dram_tensor("o_f", [B * OPLANE, 64], FP16, kind="ExternalOutput")

    with tile.TileContext(nc) as tc:
        with tc.tile_pool(name="persist", bufs=1) as pp:
            w1_sb = pp.tile([128, 6, 2, 128], BF16)
            b1_sb = pp.tile([128, 2], F32)
            w2_sb = pp.tile([128, 2, 64], BF16)
            b2_sb = pp.tile([128, 64], F32)
            x_sb = pp.tile([128, B, GRID, GRID], BF16)
            nc.sync.dma_start(w1_sb[:], w1[:])
            nc.sync.dma_start(b1_sb[:], b1[:])
            nc.sync.dma_start(w2_sb[:], w2[:])
            nc.sync.dma_start(b2_sb[:], b2[:])
            # zero everything, then land the unpadded input into the interior
            # (partition = channel, free = img,row+1,col+1).
            for img in range(B):
                nc.vector.memset(x_sb[:, img, :, :], 0.0)
            xa0 = x_sb[:]
            ps0 = xa0.ap[0][0]
            for img in range(B):
                dst_in = _rework_ap(xa0, img * NPIX + GRID + 1,
                                    [[ps0, 64], [GRID, H], [1, W]])
                src_in = bass.AP(tensor=x_in, offset=img * C * H * W,
                                 ap=[[H * W, 64], [W, H], [1, W]])
                nc.sync.dma_start(dst_in, src_in)
            # +1-shifted copy in partitions 64:127 (SWDGE; HWDGE/scalar issue
            # of this copy hard-crashes the exec unit).
            NTOT = B * NPIX                 # 67600; shift-copy [0, NTOT-1)
            sh_src = _rework_ap(xa0, 1, [[ps0, 64], [GRID, 519], [1, GRID]])
            sh_dst = _rework_ap(xa0, 64 * ps0, [[ps0, 64], [GRID, 519], [1, GRID]])
            nc.gpsimd.dma_start(sh_dst, sh_src)
            rem = 519 * GRID                # 67470: tail of 129 elems
            sh_src2 = _rework_ap(xa0, 1 + rem, [[ps0, 64], [1, NTOT - 1 - rem]])
            sh_dst2 = _rework_ap(xa0, 64 * ps0 + rem, [[ps0, 64], [1, NTOT - 1 - rem]])
            nc.gpsimd.dma_start(sh_dst2, sh_src2)

            PAIR_BASE = [-131, -1, 129]      # dy*130 - 1 for dy = -1,0,1
            with tc.tile_pool(name="conv", bufs=4) as cp, \
                 tc.tile_pool(name="slab", bufs=2) as sp, \
                 tc.tile_pool(name="cps", bufs=2, space="PSUM") as cps, \
                 tc.tile_pool(name="cps2", bufs=4, space="PSUM") as cps2:
                xa = x_sb[:]
                pstep = xa.ap[0][0]
                for img in range(B):
                    for t in range(32):
                        y0r = 4 * t
                        pbase = img * NPIX + (y0r + 1) * GRID + 1
                        f1t = []
                        for half in range(2):
                            ps = cps.tile([128, 512], F32, space="PSUM", tag="c1")
                            first = True
                            for s, db in enumerate(PAIR_BASE):
                                rhs = _rework_ap(xa, pbase + db,
                                                 [[pstep, 128], [GRID, 4], [1, 128]])
                                nc.tensor.matmul(ps[:], w1_sb[:, s, half, :], rhs,
                                                 start=first, stop=False,
                                                 skip_group_check=not first)
                                first = False
                            rhs3 = _rework_ap(xa, pbase - 129,
                                              [[pstep, 128], [GRID, 4], [1, 128]])
                            nc.tensor.matmul(ps[:], w1_sb[:, 3, half, :], rhs3,
                                             start=False, stop=False,
                                             skip_group_check=True)
                            rhs4 = _rework_ap(xa, pbase,
                                              [[pstep, 128], [GRID, 4], [1, 128]])
                            nc.tensor.matmul(ps[:], w1_sb[:, 4, half, :], rhs4,
                                             start=False, stop=False,
                                             skip_group_check=True)
                            rhs5 = _rework_ap(xa, pbase + 131,
                                              [[pstep, 128], [GRID, 4], [1, 128]])
                            nc.tensor.matmul(ps[:], w1_sb[:, 5, half, :], rhs5,
                                             start=False, stop=True,
                                             skip_group_check=True)
                            f1 = cp.tile([128, 512], BF16, tag=f"f1{half}")
                            nc.scalar.activation(f1[:], ps[:], ACTF.Relu,
                                                 bias=b1_sb[:, half:half + 1])
                            f1t.append(f1)
                        # conv2, PE-transposed: out[px, ch] per 128-px row chunk
                        slab = sp.tile([128, 4, 64], FP16, tag="slab")
                        for m in range(4):
                            ps2 = cps2.tile([128, 64], F32, space="PSUM", tag="c2")
                            nc.tensor.matmul(ps2[:], f1t[0][:, 128 * m:128 * (m + 1)],
                                             w2_sb[:, 0, :], start=True, stop=False)
                            nc.tensor.matmul(ps2[:], f1t[1][:, 128 * m:128 * (m + 1)],
                                             w2_sb[:, 1, :], start=False, stop=True,
                                             skip_group_check=True)
                            nc.vector.tensor_tensor(slab[:, m, :], ps2[:], b2_sb[:],
                                                    ALU.add)
                        dst = bass.AP(
                            tensor=o_f,
                            offset=(img * OPLANE + (y0r + 1) * OGRID + 1) * 64,
                            ap=[[64, 128], [OGRID * 64, 4], [1, 64]])
                        nc.sync.dma_start(dst, slab[:])
    _split_waits(nc)
    return nc


_SEQ_OK = ('InstUnconditionalBranch', 'InstNoOp', 'InstEventSemaphoreOp')


def _split_waits(nc, limit=1):
    """Walrus wait-slot limits: move multi-waits onto injected NoOps."""
    nid = [0]
    for f in nc.m.functions:
        for bb in f.blocks:
            il = bb.instructions
            out = []
            for ins in il:
                si = ins.sync_info
                nm = ins.__class__.__name__
                if (si is not None and len(si.on_wait) > limit
                        and nm not in _SEQ_OK):
                    waits = list(si.on_wait)
                    for k in range(0, len(waits), 1):
                        no = mybir.InstNoOp(name=f"I-wsplit{nid[0]}", ins=[], outs=[])
                        nid[0] += 1
                        no.engine = ins.engine
                        no.sync_info = mybir.SyncInfo(on_wait=waits[k:k + 1], on_update=[])
                        out.append(no)
                    ins.sync_info = mybir.SyncInfo(on_wait=[], on_update=list(si.on_update))
                out.append(ins)
            il[:] = out


def _weight_layouts(w1, b1, w2, b2):
    w1r = np.asarray(w1, np.float32).reshape(256, 64, 3, 3)

    def tapw(dy, dx):
        return w1r[:, :, dy + 1, dx + 1]             # [256, 64]
    w1_dev = np.zeros((128, 6, 2, 128), np.float32)
    pairs = [((-1, -1), (-1, 0)), ((0, -1), (0, 0)), ((1, -1), (1, 0))]
    for s, (ta, tb) in enumerate(pairs):
        for half in range(2):
            w1_dev[0:64, s, half, :] = tapw(*ta)[128 * half:128 * (half + 1)].T
            w1_dev[64:128, s, half, :] = tapw(*tb)[128 * half:128 * (half + 1)].T
    for half in range(2):
        w1_dev[0:64, 3, half, :] = tapw(-1, 1)[128 * half:128 * (half + 1)].T
        w1_dev[64:128, 4, half, :] = tapw(0, 1)[128 * half:128 * (half + 1)].T
        w1_dev[0:64, 5, half, :] = tapw(1, 1)[128 * half:128 * (half + 1)].T
    w2t = np.asarray(w2, np.float32).reshape(64, 256).T
    w2_dev = np.ascontiguousarray(np.stack([w2t[0:128], w2t[128:256]], axis=1))
    b1_dev = np.ascontiguousarray(
        np.stack([b1[0:128], b1[128:256]], 1).astype(np.float32))
    b2_dev = np.ascontiguousarray(
        np.broadcast_to(np.asarray(b2, np.float32)[None, :], (128, 64)))
    return (w1_dev.astype(BF), b1_dev, w2_dev.astype(BF), b2_dev)


def _get_rt():
    rt = _cache.get('rt')
    if rt is not None:
        return rt
    install_neuronx_cc_hook()
    nc = build_nc()
    partition_name = nc.partition_id_tensor.name if nc.partition_id_tensor else None
    in_names, out_names, out_avals, zero_shapes = [], [], [], []
    for alloc in nc.m.functions[0].allocations:
        if not isinstance(alloc, mybir.MemoryLocationSet):
            continue
        name = alloc.memorylocations[0].name
        if alloc.kind == "ExternalInput":
            if name != partition_name:
                in_names.append(name)
        elif alloc.kind == "ExternalOutput":
            shape = tuple(alloc.tensor_shape)
            dtype = mybir.dt.np(alloc.dtype)
            out_names.append(name)
            out_avals.append(jax.core.ShapedArray(shape, dtype))
            zero_shapes.append((shape, dtype))
    n_params = len(in_names)
    n_outs = len(out_avals)
    in_names_all = in_names + out_names + ([partition_name] if partition_name else [])
    donate = tuple(range(n_params, n_params + n_outs))

    def _body(*args):
        operands = list(args)
        if partition_name is not None:
            operands.append(partition_id_tensor())
        outs = _bass_exec_p.bind(
            *operands, out_avals=tuple(out_avals),
            in_names=tuple(in_names_all), out_names=tuple(out_names),
            lowering_input_output_aliases=(), sim_require_finite=True,
            sim_require_nnan=True, nc=nc)
        return tuple(outs)

    dev0 = jax.devices()[0]
    runf = jax.jit(_body, donate_argnums=donate, keep_unused=True)
    zeros_fns = [
        jax.jit(lambda s=s, dt=dt: jnp.zeros(s, dt), device=dev0)
        for s, dt in zero_shapes]

    cpu = jax.devices('cpu')[0]

    @partial(jax.jit, device=cpu)
    def comb(F2, y0sel, y1sel, wx, wy):
        F2 = F2.astype(jnp.float32)
        p00 = jnp.take(F2, y0sel, axis=0)
        p01 = jnp.take(F2, y0sel + 1, axis=0)
        p10 = jnp.take(F2, y1sel, axis=0)
        p11 = jnp.take(F2, y1sel + 1, axis=0)
        return ((1 - wy) * ((1 - wx) * p00 + wx * p01)
                + wy * ((1 - wx) * p10 + wx * p11))

    rt = dict(nc=nc, in_names=in_names, out_names=out_names, runf=runf,
              zeros_fns=zeros_fns, dev0=dev0, comb=comb)
    _cache['rt'] = rt
    return rt


def _dev_weights(rt, inputs):
    """Device-resident weight arrays, re-uploaded only when the bytes change."""
    w1 = np.asarray(inputs['conv1_w'], np.float32)
    b1 = np.asarray(inputs['conv1_b'], np.float32)
    w2 = np.asarray(inputs['conv2_w'], np.float32)
    b2 = np.asarray(inputs['conv2_b'], np.float32)
    cached = _cache.get('wts')
    if cached is not None:
        ow1, ob1, ow2, ob2, dev = cached
        if (np.array_equal(w1, ow1) and np.array_equal(b1, ob1)
                and np.array_equal(w2, ow2) and np.array_equal(b2, ob2)):
            return dev
    lay = _weight_layouts(w1, b1, w2, b2)
    dev = {name: jax.device_put(arr, rt['dev0'])
           for name, arr in zip(['w1', 'b1', 'w2', 'b2'], lay)}
    for a in dev.values():
        a.block_until_ready()
    _cache['wts'] = (w1.copy(), b1.copy(), w2.copy(), b2.copy(), dev)
    return dev


def kernel(**inputs):
    rt = _get_rt()
    dev_w = _dev_weights(rt, inputs)

    fw = np.asarray(inputs['fuse_w'], np.float32)
    pw = np.asarray(inputs['poly_w'], np.float32)
    cw = _cache.get('Wf2')
    if cw is None or not (np.array_equal(fw, cw[0]) and np.array_equal(pw, cw[1])):
        Wf = (fw @ pw).T                               # (8256, 256) rows c*129+j
        Wf2 = np.ascontiguousarray(
            Wf.reshape(64, 129, 256).transpose(1, 0, 2).reshape(129 * 64, 256))
        _cache['Wf2'] = (fw.copy(), pw.copy(), Wf2)
    Wf2 = _cache['Wf2'][2]

    x16 = np.asarray(inputs['cnn_feature'], np.float32).astype(BF)
    zeros = [fn() for fn in rt['zeros_fns']]
    out_arrs = rt['runf'](x16, dev_w['w1'], dev_w['b1'], dev_w['w2'],
                          dev_w['b2'], *zeros)         # async dispatch

    # ---- host work overlapped with device exec ----
    wh = np.asarray(inputs['wh_pred'], np.float32)
    ct_ind = np.asarray(inputs['ct_ind'], np.int64)
    ct_img = np.asarray(inputs['ct_img_idx'], np.int64)
    N = ct_ind.shape[0]
    ctx = (ct_ind % W).astype(np.float32)
    cty = (ct_ind // W).astype(np.float32)
    whr = wh[ct_img, :, ct_ind // W, ct_ind % W]       # (N, 2P)
    ct4 = np.stack([ctx, cty], -1) * 4.0               # (N,2)
    init = whr.reshape(N, P, 2) * 40.0 + ct4[:, None, :]
    ct = np.stack([ctx, cty], -1)
    points = np.concatenate([ct[:, None, :], init / 4.0], axis=1)  # (N,129,2)
    fb = np.asarray(inputs['fuse_b'], np.float32)

    x = points[..., 0] - 0.5
    y = points[..., 1] - 0.5
    x0 = np.floor(x)
    y0 = np.floor(y)
    wx = (x - x0).astype(np.float32)[..., None]
    wy = (y - y0).astype(np.float32)[..., None]
    x0i = x0.astype(np.int32)
    y0i = y0.astype(np.int32)
    # padded col of the left neighbor; (129,130) is an all-zero pair, used for
    # fully-OOB x. rows: plain clip works (rows 0 and 129 are both zero).
    xsel = np.where(x0i >= -1, np.minimum(x0i + 1, 129), 129)
    ybase = ct_img.astype(np.int32)[:, None] * 130
    y0sel = (ybase + np.clip(y0i + 1, 0, 129)) * OGRID + xsel
    y1sel = (ybase + np.clip(y0i + 2, 0, 129)) * OGRID + xsel

    # ---- collect f (4 padded planes, zero borders via donated zeros) ----
    o_f = np.asarray(out_arrs[0])                      # (B*OPLANE, 64) fp16
    fp = np.asarray(rt['comb'](o_f, y0sel, y1sel, wx, wy))   # (N,129,64) f32
    offsets = fp.reshape(N, 129 * 64) @ Wf2 + fb
    coar = offsets.reshape(N, P, 2) * 16.0 + init
    return init, coar


# revision 8
# speedup vs baseline: 8.2624x; 1.5051x over previous
"""Trainium2 Bass kernel for nn_Decode (CenterNet-style polygon decode).

Single NeuronCore does the conv stack for all 4 images: conv3x3(64->256)+relu
-> conv1x1(256->64) in bf16 on the PE. The axon link (~60-75MB/s, ~50ms fixed
per exec RPC) is the bottleneck, not compute (<1ms on PE), so the design
minimizes bytes moved and RPC count:
  - input goes up unpadded as one contiguous bf16 cast of cnn_feature (8.4MB);
    zero-padding happens on device (memset + strided DMA into SBUF).
  - conv1 uses the shift-pair trick (3 K=128 pair matmuls + 3 zero-padded tap
    matmuls per 512-px PSUM tile, stationary weights); the +1-shifted input
    copy is built on device via an SWDGE SBUF->SBUF DMA (HWDGE/scalar issue
    of the same copy hard-crashes the exec unit).
  - conv2 is PE-transposed (f1 chunk stationary, w2 moving) so PSUM comes out
    [px, ch] and lands in DRAM as 4 zero-padded fp16 planes (130x131, 1px
    left / 2px right pad) -- the host bilinear then needs no transpose and no
    validity masks (zero border == padding_mode=zeros), 8.7MB down.
  - weights are cached on device across calls (byte-compared); donated output
    zero-buffers are created on device, never uploaded.
Host: init-poly math (overlapped with device exec), fused XLA-CPU pair-gather
bilinear off the padded planes, refine matmul via fused (fuse_w@poly_w)
reordered j-major so no transpose of the sampled features is needed.
"""
import sys
sys.path.insert(0, '/opt/trn_rl_repo')
import numpy as np
import ml_dtypes
from functools import partial

import jax
import jax.numpy as jnp

import concourse.bass as bass
import concourse.mybir as mybir
import concourse.tile as tile
from concourse.bass2jax import _bass_exec_p, partition_id_tensor, install_neuronx_cc_hook

F32 = mybir.dt.float32
BF16 = mybir.dt.bfloat16
FP16 = mybir.dt.float16
FP8 = mybir.dt.float8e4
ALU = mybir.AluOpType
ACTF = mybir.ActivationFunctionType
BF = ml_dtypes.bfloat16
F8 = ml_dtypes.float8_e4m3

P = 128
B, C, H, W = 4, 64, 128, 128
GRID = 130                 # padded input plane width/height
NPIX = GRID * GRID         # 16900 input px per image
OGRID = 131                # output plane width (1 left pad, 2 right pad)
OPLANE = 130 * OGRID       # 17030 output rows per image
_cache = {}


def _rework_ap(base_ap, extra_off, dims):
    return bass.AP(tensor=base_ap.tensor, offset=base_ap.offset + extra_off, ap=dims)


def build_nc():
    nc = bass.Bass()
    x_in = nc.dram_tensor("x_in", [B, C, H, W], FP8, kind="ExternalInput")
    w1 = nc.dram_tensor("w1", [128, 6, 2, 128], FP8, kind="ExternalInput")
    b1 = nc.dram_tensor("b1", [128, 2], F32, kind="ExternalInput")
    w2 = nc.dram_tensor("w2", [128, 2, 64], BF16, kind="ExternalInput")
    b2 = nc.dram_tensor("b2", [128, 64], F32, kind="ExternalInput")
    o_f = nc.dram_tensor("o_f", [B * OPLANE, 64], FP8, kind="ExternalOutput")

    with tile.TileContext(nc) as tc:
        with tc.tile_pool(name="persist", bufs=1) as pp:
            w1_sb = pp.tile([128, 6, 2, 128], FP8)
            b1_sb = pp.tile([128, 2], F32)
            w2_sb = pp.tile([128, 2, 64], BF16)
            b2_sb = pp.tile([128, 64], F32)
            x_sb = pp.tile([128, B, GRID, GRID], FP8)
            nc.sync.dma_start(w1_sb[:], w1[:])
            nc.sync.dma_start(b1_sb[:], b1[:])
            nc.sync.dma_start(w2_sb[:], w2[:])
            nc.sync.dma_start(b2_sb[:], b2[:])
            # zero everything, then land the unpadded input into the interior
            # (partition = channel, free = img,row+1,col+1).
            for img in range(B):
                nc.vector.memset(x_sb[:, img, :, :], 0.0)
            xa0 = x_sb[:]
            ps0 = xa0.ap[0][0]
            for img in range(B):
                dst_in = _rework_ap(xa0, img * NPIX + GRID + 1,
                                    [[ps0, 64], [GRID, H], [1, W]])
                src_in = bass.AP(tensor=x_in, offset=img * C * H * W,
                                 ap=[[H * W, 64], [W, H], [1, W]])
                nc.sync.dma_start(dst_in, src_in)
            # +1-shifted copy in partitions 64:127 (SWDGE; HWDGE/scalar issue
            # of this copy hard-crashes the exec unit).
            NTOT = B * NPIX                 # 67600; shift-copy [0, NTOT-1)
            sh_src = _rework_ap(xa0, 1, [[ps0, 64], [GRID, 519], [1, GRID]])
            sh_dst = _rework_ap(xa0, 64 * ps0, [[ps0, 64], [GRID, 519], [1, GRID]])
            nc.gpsimd.dma_start(sh_dst, sh_src)
            rem = 519 * GRID                # 67470: tail of 129 elems
            sh_src2 = _rework_ap(xa0, 1 + rem, [[ps0, 64], [1, NTOT - 1 - rem]])
            sh_dst2 = _rework_ap(xa0, 64 * ps0 + rem, [[ps0, 64], [1, NTOT - 1 - rem]])
            nc.gpsimd.dma_start(sh_dst2, sh_src2)

            PAIR_BASE = [-131, -1, 129]      # dy*130 - 1 for dy = -1,0,1
            with tc.tile_pool(name="conv", bufs=4) as cp, \
                 tc.tile_pool(name="slab", bufs=2) as sp, \
                 tc.tile_pool(name="cps", bufs=2, space="PSUM") as cps, \
                 tc.tile_pool(name="cps2", bufs=4, space="PSUM") as cps2:
                xa = x_sb[:]
                pstep = xa.ap[0][0]
                for img in range(B):
                    for t in range(32):
                        y0r = 4 * t
                        pbase = img * NPIX + (y0r + 1) * GRID + 1
                        f1t = []
                        for half in range(2):
                            ps = cps.tile([128, 512], F32, space="PSUM", tag="c1")
                            first = True
                            for s, db in enumerate(PAIR_BASE):
                                rhs = _rework_ap(xa, pbase + db,
                                                 [[pstep, 128], [GRID, 4], [1, 128]])
                                nc.tensor.matmul(ps[:], w1_sb[:, s, half, :], rhs,
                                                 start=first, stop=False,
                                                 skip_group_check=not first)
                                first = False
                            rhs3 = _rework_ap(xa, pbase - 129,
                                              [[pstep, 128], [GRID, 4], [1, 128]])
                            nc.tensor.matmul(ps[:], w1_sb[:, 3, half, :], rhs3,
                                             start=False, stop=False,
                                             skip_group_check=True)
                            rhs4 = _rework_ap(xa, pbase,
                                              [[pstep, 128], [GRID, 4], [1, 128]])
                            nc.tensor.matmul(ps[:], w1_sb[:, 4, half, :], rhs4,
                                             start=False, stop=False,
                                             skip_group_check=True)
                            rhs5 = _rework_ap(xa, pbase + 131,
                                              [[pstep, 128], [GRID, 4], [1, 128]])
                            nc.tensor.matmul(ps[:], w1_sb[:, 5, half, :], rhs5,
                                             start=False, stop=True,
                                             skip_group_check=True)
                            f1 = cp.tile([128, 512], BF16, tag=f"f1{half}")
                            nc.scalar.activation(f1[:], ps[:], ACTF.Relu,
                                                 bias=b1_sb[:, half:half + 1],
                                                 scale=1.0 / 64.0)
                            f1t.append(f1)
                        # conv2, PE-transposed: out[px, ch] per 128-px row chunk
                        slab = sp.tile([128, 4, 64], FP8, tag="slab")
                        for m in range(4):
                            ps2 = cps2.tile([128, 64], F32, space="PSUM", tag="c2")
                            nc.tensor.matmul(ps2[:], f1t[0][:, 128 * m:128 * (m + 1)],
                                             w2_sb[:, 0, :], start=True, stop=False)
                            nc.tensor.matmul(ps2[:], f1t[1][:, 128 * m:128 * (m + 1)],
                                             w2_sb[:, 1, :], start=False, stop=True,
                                             skip_group_check=True)
                            nc.vector.tensor_tensor(slab[:, m, :], ps2[:], b2_sb[:],
                                                    ALU.add)
                        dst = bass.AP(
                            tensor=o_f,
                            offset=(img * OPLANE + (y0r + 1) * OGRID + 1) * 64,
                            ap=[[64, 128], [OGRID * 64, 4], [1, 64]])
                        nc.sync.dma_start(dst, slab[:])
    _split_waits(nc)
    return nc


_SEQ_OK = ('InstUnconditionalBranch', 'InstNoOp', 'InstEventSemaphoreOp')


def _split_waits(nc, limit=1):
    """Walrus wait-slot limits: move multi-waits onto injected NoOps."""
    nid = [0]
    for f in nc.m.functions:
        for bb in f.blocks:
            il = bb.instructions
            out = []
            for ins in il:
                si = ins.sync_info
                nm = ins.__class__.__name__
                if (si is not None and len(si.on_wait) > limit
                        and nm not in _SEQ_OK):
                    waits = list(si.on_wait)
                    for k in range(0, len(waits), 1):
                        no = mybir.InstNoOp(name=f"I-wsplit{nid[0]}", ins=[], outs=[])
                        nid[0] += 1
                        no.engine = ins.engine
                        no.sync_info = mybir.SyncInfo(on_wait=waits[k:k + 1], on_update=[])
                        out.append(no)
                    ins.sync_info = mybir.SyncInfo(on_wait=[], on_update=list(si.on_update))
                out.append(ins)
            il[:] = out


def _weight_layouts(w1, b1, w2, b2):
    w1r = np.asarray(w1, np.float32).reshape(256, 64, 3, 3)

    def tapw(dy, dx):
        return w1r[:, :, dy + 1, dx + 1]             # [256, 64]
    w1_dev = np.zeros((128, 6, 2, 128), np.float32)
    pairs = [((-1, -1), (-1, 0)), ((0, -1), (0, 0)), ((1, -1), (1, 0))]
    for s, (ta, tb) in enumerate(pairs):
        for half in range(2):
            w1_dev[0:64, s, half, :] = tapw(*ta)[128 * half:128 * (half + 1)].T
            w1_dev[64:128, s, half, :] = tapw(*tb)[128 * half:128 * (half + 1)].T
    for half in range(2):
        w1_dev[0:64, 3, half, :] = tapw(-1, 1)[128 * half:128 * (half + 1)].T
        w1_dev[64:128, 4, half, :] = tapw(0, 1)[128 * half:128 * (half + 1)].T
        w1_dev[0:64, 5, half, :] = tapw(1, 1)[128 * half:128 * (half + 1)].T
    w2t = np.asarray(w2, np.float32).reshape(64, 256).T
    w2_dev = np.ascontiguousarray(np.stack([w2t[0:128], w2t[128:256]], axis=1))
    b1_dev = np.ascontiguousarray(
        np.stack([b1[0:128], b1[128:256]], 1).astype(np.float32))
    b2_dev = np.ascontiguousarray(
        np.broadcast_to(np.asarray(b2, np.float32)[None, :], (128, 64)))
    return ((w1_dev * 64.0).astype(F8), b1_dev, w2_dev.astype(BF), b2_dev)


def _get_rt():
    rt = _cache.get('rt')
    if rt is not None:
        return rt
    install_neuronx_cc_hook()
    nc = build_nc()
    partition_name = nc.partition_id_tensor.name if nc.partition_id_tensor else None
    in_names, out_names, out_avals, zero_shapes = [], [], [], []
    for alloc in nc.m.functions[0].allocations:
        if not isinstance(alloc, mybir.MemoryLocationSet):
            continue
        name = alloc.memorylocations[0].name
        if alloc.kind == "ExternalInput":
            if name != partition_name:
                in_names.append(name)
        elif alloc.kind == "ExternalOutput":
            shape = tuple(alloc.tensor_shape)
            dtype = mybir.dt.np(alloc.dtype)
            out_names.append(name)
            out_avals.append(jax.core.ShapedArray(shape, dtype))
            zero_shapes.append((shape, dtype))
    n_params = len(in_names)
    n_outs = len(out_avals)
    in_names_all = in_names + out_names + ([partition_name] if partition_name else [])
    donate = tuple(range(n_params, n_params + n_outs))

    def _body(*args):
        operands = list(args)
        if partition_name is not None:
            operands.append(partition_id_tensor())
        outs = _bass_exec_p.bind(
            *operands, out_avals=tuple(out_avals),
            in_names=tuple(in_names_all), out_names=tuple(out_names),
            lowering_input_output_aliases=(), sim_require_finite=True,
            sim_require_nnan=True, nc=nc)
        return tuple(outs)

    dev0 = jax.devices()[0]
    runf = jax.jit(_body, donate_argnums=donate, keep_unused=True)
    zeros_fns = [
        jax.jit(lambda s=s, dt=dt: jnp.zeros(s, dt), device=dev0)
        for s, dt in zero_shapes]

    cpu = jax.devices('cpu')[0]

    @partial(jax.jit, device=cpu)
    def comb(F2, y0sel, y1sel, wx, wy):
        F2 = F2.astype(jnp.float32)
        p00 = jnp.take(F2, y0sel, axis=0)
        p01 = jnp.take(F2, y0sel + 1, axis=0)
        p10 = jnp.take(F2, y1sel, axis=0)
        p11 = jnp.take(F2, y1sel + 1, axis=0)
        return ((1 - wy) * ((1 - wx) * p00 + wx * p01)
                + wy * ((1 - wx) * p10 + wx * p11))

    rt = dict(nc=nc, in_names=in_names, out_names=out_names, runf=runf,
              zeros_fns=zeros_fns, dev0=dev0, comb=comb)
    _cache['rt'] = rt
    return rt


def _dev_weights(rt, inputs):
    """Device-resident weight arrays, re-uploaded only when the bytes change."""
    w1 = np.asarray(inputs['conv1_w'], np.float32)
    b1 = np.asarray(inputs['conv1_b'], np.float32)
    w2 = np.asarray(inputs['conv2_w'], np.float32)
    b2 = np.asarray(inputs['conv2_b'], np.float32)
    cached = _cache.get('wts')
    if cached is not None:
        ow1, ob1, ow2, ob2, dev = cached
        if (np.array_equal(w1, ow1) and np.array_equal(b1, ob1)
                and np.array_equal(w2, ow2) and np.array_equal(b2, ob2)):
            return dev
    lay = _weight_layouts(w1, b1, w2, b2)
    dev = {name: jax.device_put(arr, rt['dev0'])
           for name, arr in zip(['w1', 'b1', 'w2', 'b2'], lay)}
    for a in dev.values():
        a.block_until_ready()
    _cache['wts'] = (w1.copy(), b1.copy(), w2.copy(), b2.copy(), dev)
    return dev


def kernel(**inputs):
    rt = _get_rt()
    dev_w = _dev_weights(rt, inputs)

    fw = np.asarray(inputs['fuse_w'], np.float32)
    pw = np.asarray(inputs['poly_w'], np.float32)
    cw = _cache.get('Wf2')
    if cw is None or not (np.array_equal(fw, cw[0]) and np.array_equal(pw, cw[1])):
        Wf = (fw @ pw).T                               # (8256, 256) rows c*129+j
        Wf2 = np.ascontiguousarray(
            Wf.reshape(64, 129, 256).transpose(1, 0, 2).reshape(129 * 64, 256))
        _cache['Wf2'] = (fw.copy(), pw.copy(), Wf2)
    Wf2 = _cache['Wf2'][2]

    x16 = np.asarray(inputs['cnn_feature'], np.float32).astype(F8)
    zeros = [fn() for fn in rt['zeros_fns']]
    out_arrs = rt['runf'](x16, dev_w['w1'], dev_w['b1'], dev_w['w2'],
                          dev_w['b2'], *zeros)         # async dispatch

    # ---- host work overlapped with device exec ----
    wh = np.asarray(inputs['wh_pred'], np.float32)
    ct_ind = np.asarray(inputs['ct_ind'], np.int64)
    ct_img = np.asarray(inputs['ct_img_idx'], np.int64)
    N = ct_ind.shape[0]
    ctx = (ct_ind % W).astype(np.float32)
    cty = (ct_ind // W).astype(np.float32)
    whr = wh[ct_img, :, ct_ind // W, ct_ind % W]       # (N, 2P)
    ct4 = np.stack([ctx, cty], -1) * 4.0               # (N,2)
    init = whr.reshape(N, P, 2) * 40.0 + ct4[:, None, :]
    ct = np.stack([ctx, cty], -1)
    points = np.concatenate([ct[:, None, :], init / 4.0], axis=1)  # (N,129,2)
    fb = np.asarray(inputs['fuse_b'], np.float32)

    x = points[..., 0] - 0.5
    y = points[..., 1] - 0.5
    x0 = np.floor(x)
    y0 = np.floor(y)
    wx = (x - x0).astype(np.float32)[..., None]
    wy = (y - y0).astype(np.float32)[..., None]
    x0i = x0.astype(np.int32)
    y0i = y0.astype(np.int32)
    # padded col of the left neighbor; (129,130) is an all-zero pair, used for
    # fully-OOB x. rows: plain clip works (rows 0 and 129 are both zero).
    xsel = np.where(x0i >= -1, np.minimum(x0i + 1, 129), 129)
    ybase = ct_img.astype(np.int32)[:, None] * 130
    y0sel = (ybase + np.clip(y0i + 1, 0, 129)) * OGRID + xsel
    y1sel = (ybase + np.clip(y0i + 2, 0, 129)) * OGRID + xsel

    # ---- collect f (4 padded planes, zero borders via donated zeros) ----
    o_f = np.asarray(out_arrs[0])                      # (B*OPLANE, 64) fp16
    fp = np.asarray(rt['comb'](o_f, y0sel, y1sel, wx, wy))   # (N,129,64) f32
    offsets = fp.reshape(N, 129 * 64) @ Wf2 + fb
    coar = offsets.reshape(N, P, 2) * 16.0 + init
    return init, coar
